# revision 32
# baseline (speedup 1.0000x reference)
"""Trainium2 Bass kernel for CGNN message-passing ODE (nn_CGNN_51333449121989).

Math: the reference integrates the affine ODE z' = diag(sigmoid(alpha))*0.5*(A z - z) + x0
with RK4 (4 steps, dt=0.25) from z0 = x0, where x0 = [x @ m1_w + m1_b, zeros].
Since each RK4 step is the affine map z <- P(M) z + Q(M) x0 with
M = diag(a)*0.5*(A - I), the final state is an exact degree-16 polynomial
R(M) x0, evaluated here by 16 Horner iterations:
    y <- a05 * (A y - y) + r_k * x0      (a05 = 0.5*sigmoid(alpha))
Feature columns H..2H-1 of the state are identically zero (columns evolve
independently and start/force at zero), so the working state is [N, H].

Distribution: 1D node partition over 8 cores (6250 rows each, padded to
6272 = 49*128).  Each core owns the edges whose src falls in its row range.
The replica of y lives as bf16 "pair tokens" (node 2t and 2t+1 concatenated,
128 bf16 = 256 B per token, 25088 tokens < int16 range) packed 128 tokens
per stripe in SBUF.  Per iteration each core:
  - AllGathers the bf16 replica to HBM, copies it into SBUF with one
    full-bandwidth DMA ([128, 50176 B], contiguous per partition),
  - dma_gather's tokens[dst//2] from SBUF with transpose=True (SBUF-source
    gathers do not pay the HBM small-descriptor penalty); the gather output
    is feature-major [128 feat, edges],
  - per 128-edge chunk: PE-transposes the chunk back to edge-major, then
    segment-sums with two parity-split PE matmuls:
    psum[128 rows, H] += W_even^T @ msg[:, 0:64] + W_odd^T @ msg[:, 64:128],
    where W_par[e, r] = (r == src_local[e]) * w_e * (dst_e parity == par)
    is built on the vector engine in bf16 via a dual-op tensor_scalar,
  - applies the Horner update (f32 locally) and publishes its bf16 shard.

I/O engineering (the wall-clock is dominated by the axon tunnel, ~40 MB/s):
  - x (the 100 MB input) and m1_w ship as bf16; PE matmuls run bf16.
  - gather index tables ship untiled ([16, n]) and are replicated to the
    128-partition layout on device; src-row table ships as uint8 and edge
    weights as bf16 (cast on device).
  - the output ships back as bf16 and is upcast on host.
  - run_spmd() caches the lowered executable and the device-resident input
    buffers across calls; donated output zero-buffers are created on device
    inside the same jit; output shards are fetched in parallel.
"""

import sys

sys.path.insert(0, "/opt/trn_rl_repo")

from concurrent.futures import ThreadPoolExecutor
from dataclasses import dataclass

import numpy as np
import ml_dtypes

BF16 = np.dtype(ml_dtypes.bfloat16)


# ---------------------------------------------------------------- constants
@dataclass(frozen=True)
class Cfg:
    N: int = 50000          # nodes
    E: int = 600000         # edges
    F: int = 500            # input features
    H: int = 64             # hidden (ODE state width)
    C: int = 40             # classes
    NCORES: int = 8
    NITER: int = 16         # Horner iterations (degree-16 polynomial, exact)
    DT: float = 0.25        # T / STEPS from the reference
    GCH: int = 48           # gather-group size in chunks (48*128 idx per call)

    @property
    def NSH(self):          # true rows per core
        return self.N // self.NCORES

    @property
    def BLOCKS(self):       # 128-row blocks per core
        return (self.NSH + 127) // 128

    @property
    def NLOC(self):         # padded rows per core
        return self.BLOCKS * 128

    @property
    def NREP(self):         # replica rows
        return self.NCORES * self.NLOC

    @property
    def HALF(self):         # low/high split of replica rows (int16 gather idx)
        return self.NREP // 2


def horner_coeffs(cfg: Cfg) -> np.ndarray:
    """Coefficients r_0..r_16 of the exact RK4 polynomial R(M)."""
    dt = cfg.DT
    deg = max(cfg.NITER, 16)
    P = np.zeros(deg + 1)
    Q = np.zeros(deg + 1)
    P[0] = 1.0
    fact = 1.0
    for j in range(1, 5):
        fact *= j
        P[j] = dt**j / fact
        Q[j - 1] = dt**j / fact

    def pmul(a, b):
        out = np.zeros(2 * deg + 1)
        for i in range(deg + 1):
            if a[i]:
                out[i : i + deg + 1] += a[i] * b
        return out[: deg + 1]

    P2 = pmul(P, P)
    P3 = pmul(P2, P)
    P4 = pmul(P3, P)
    S = P3 + P2 + P
    S[0] += 1.0
    R = P4 + pmul(S, Q)
    return R


# ------------------------------------------------------------ tile patch
def _patch_tile_drain():
    """This toolchain's walrus rejects instructions with several sem waits;
    split TileContext's exit-drain waits across single-wait nops."""
    import concourse.tile as tile
    from concourse.vector_clock import ScopedClock
    from bass_rust import VectorClock

    if getattr(tile.TileContext, "_drain_patched", False):
        return

    def _drain_and_barrier(self, tick_clock, wait_clock):
        gc = tick_clock.global_clock
        scoped = ScopedClock({None: gc})
        for scope, vc in scoped.items():
            procs = [i for i in range(len(vc)) if vc[i] > 0]
            for p in procs:
                pvc = VectorClock()
                pvc.require_at_least(p, vc[p])
                nop = self.nc.sync.nop(nofuse=True, hint="drain_split")
                wait_clock.add_sem_waits(nop.ins, ScopedClock({scope: pvc}))
        self.nc.sync.drain()
        self.nc.all_engine_barrier()
        assert self.sems is not None
        popped = self.nc._tile_sem_poison_stack.pop()
        assert popped is self._sem_poison
        self.nc.clear_and_free_semaphores(list(self.sems.allocated().values()))
        self.nc.all_engine_barrier()

    tile.TileContext._drain_and_barrier = _drain_and_barrier
    tile.TileContext._drain_patched = True


# ------------------------------------------------------------ host prep
@dataclass
class Plan:
    # uniform chunk structure
    nch: int
    cbs: np.ndarray           # [BLOCKS] chunks per block
    ngrp: int
    # per-core packed tensors
    in_maps: list


def build_plan(cfg: Cfg, inputs: dict) -> Plan:
    x = np.asarray(inputs["x"], np.float32)
    ew = np.asarray(inputs["edge_w"], np.float32)
    src = np.asarray(inputs["edge_src"], np.int64)
    dst = np.asarray(inputs["edge_dst"], np.int64)
    m1w = np.asarray(inputs["m1_w"], np.float32)
    m1b = np.asarray(inputs["m1_b"], np.float32)
    alpha = np.asarray(inputs["alpha_train"], np.float32)
    m2w = np.asarray(inputs["m2_w"], np.float32)
    m2b = np.asarray(inputs["m2_b"], np.float32)

    NC, NSH, NLOC, BLOCKS = cfg.NCORES, cfg.NSH, cfg.NLOC, cfg.BLOCKS
    GCH = cfg.GCH
    NSTRIPES = cfg.NREP // 2 // 128               # token stripes (196)

    owner = src // NSH
    owner = np.minimum(owner, NC - 1)
    src_loc = src - owner * NSH
    downer = dst // NSH
    downer = np.minimum(downer, NC - 1)
    dpos = downer * NLOC + (dst - downer * NSH)   # replica row of dst
    tok = dpos // 2                               # pair token id
    par = dpos % 2                                # which half of the token
    # SBUF token placement: token t -> partition t // NSTRIPES,
    # stripe t % NSTRIPES.  The gather addresses token i as
    # (partition i % 128, stripe i // 128), so remap:
    tok_idx = (tok % NSTRIPES) * 128 + tok // NSTRIPES
    block = src_loc // 128
    srow = src_loc % 128                          # row within block

    # ---- per-(core, block) edge buckets
    counts = np.zeros((NC, BLOCKS), np.int64)
    np.add.at(counts, (owner, block), 1)
    cbs = np.ceil(counts.max(axis=0) / 128).astype(np.int64)   # [BLOCKS]
    cbs = np.maximum(cbs, 1)                # every block needs >=1 chunk
    nch = int(cbs.sum())
    ngrp = (nch + GCH - 1) // GCH

    off = np.concatenate([[0], np.cumsum(cbs)])

    KP = ((cfg.F + 1 + 127) // 128) * 128
    m1w_aug = np.zeros((KP, cfg.H), np.float32)
    m1w_aug[: cfg.F] = m1w
    m1w_aug[cfg.F] = m1b
    m1w_aug = m1w_aug.astype(BF16)

    m2w_aug = np.zeros((cfg.H + 1, cfg.C), np.float32)
    m2w_aug[: cfg.H] = m2w
    m2w_aug[cfg.H] = m2b

    iota = np.tile(np.arange(128, dtype=np.float32), (128, 1))
    ident = np.eye(128, dtype=np.float32)

    # sort edges per core by (block, token) for gather locality
    in_maps = []
    for c in range(NC):
        sel = owner == c
        eb, er, et, ep, ewc = (
            block[sel], srow[sel], tok_idx[sel], par[sel], ew[sel])

        src_tab = np.zeros((128, nch), np.uint8)
        we_tab = np.zeros((128, nch), np.float32)   # even-parity weights
        wo_tab = np.zeros((128, nch), np.float32)   # odd-parity weights
        idx_arr = np.zeros(nch * 128, np.int64)

        order = np.lexsort((et, eb))
        b_s, r_s, t_s, p_s, w_s = (
            eb[order], er[order], et[order], ep[order], ewc[order])
        # place edges of block b into its chunk range [off[b], off[b+1])
        starts = np.searchsorted(b_s, np.arange(BLOCKS))
        ends = np.searchsorted(b_s, np.arange(BLOCKS), side="right")
        for b in range(BLOCKS):
            n_edges = ends[b] - starts[b]
            pos0 = off[b] * 128
            sl = slice(starts[b], ends[b])
            idx_arr[pos0 : pos0 + n_edges] = t_s[sl]
            cols = np.arange(n_edges) // 128 + off[b]
            rows = np.arange(n_edges) % 128
            src_tab[rows, cols] = r_s[sl]
            wvals = np.where(p_s[sl] == 0, w_s[sl], 0.0)
            we_tab[rows, cols] = wvals
            wo_tab[rows, cols] = w_s[sl] - wvals
            # padding edges keep w=0 / idx=0 / src_row=0

        n_full = ngrp * GCH * 128
        full = np.zeros(n_full, np.int64)
        full[: len(idx_arr)] = idx_arr
        idx_w = full.reshape(-1, 16).T.astype(np.int16)       # [16, n/16]

        # encoder input: per block a [128, KP] tile where
        # xpack[b, p, kc*128 + n] = x_aug[b*128 + n, kc*128 + p]
        rows = slice(c * NSH, (c + 1) * NSH)
        xsh = np.zeros((NLOC, KP), np.float32)
        xsh[:NSH, : cfg.F] = x[rows]
        xsh[:NSH, cfg.F] = 1.0                     # bias column
        # [NLOC, KP] -> [BLOCKS, 128n, KCH, 128p] -> [BLOCKS, 128p, KCH*128n]
        KCH = KP // 128
        xpack = (
            xsh.reshape(BLOCKS, 128, KCH, 128)
            .transpose(0, 3, 2, 1)
            .reshape(BLOCKS, 128, KP)
            .astype(BF16)
        )

        al = np.zeros(NLOC, np.float32)
        al[:NSH] = alpha[rows]
        alpha_s = al.reshape(BLOCKS, 128).T.copy()      # [128, BLOCKS]

        in_maps.append(
            dict(
                xpack=np.ascontiguousarray(xpack), m1w=m1w_aug, m2w=m2w_aug,
                alpha_s=alpha_s, iota=iota, ident=ident,
                srct=src_tab, we=we_tab.astype(BF16), wo=wo_tab.astype(BF16),
                idx=np.ascontiguousarray(idx_w),
            )
        )

    return Plan(nch, np.asarray(cbs), ngrp, in_maps)


# ------------------------------------------------------------ device program
def build_program(cfg: Cfg, plan: Plan, rcoef: np.ndarray,
                  timing_mode: bool = False, phases: str = "ehda",
                  reps=(1, 1, 1)):
    RG, RM, RU = reps   # timing: repeat gathers / matmuls / updates
    """timing_mode: single-core variant for TimelineSim (collectives replaced
    by a local DMA of the same local traffic)."""
    import concourse.bacc as bacc
    import concourse.mybir as mybir
    import concourse.tile as tile

    _patch_tile_drain()

    NC, H, BLOCKS, NLOC, NREP = (
        cfg.NCORES, cfg.H, cfg.BLOCKS, cfg.NLOC, cfg.NREP)
    GCH = cfg.GCH
    NTOK = NREP // 2                 # pair tokens in the replica
    KP = ((cfg.F + 1 + 127) // 128) * 128
    KCH = KP // 128
    f32 = mybir.dt.float32
    bf16 = mybir.dt.bfloat16

    nc = bacc.Bacc("TRN2", target_bir_lowering=False, debug=False,
                   num_devices=1 if timing_mode else NC)

    def allgather(ins, outs):
        if "a" not in phases:
            return
        if timing_mode:
            # local-cost stand-in: write own shard into the replica
            nc.sync.dma_start(
                out=outs[0][0:16, :], in_=ins[0].rearrange("(p x) f -> p (x f)", p=16))
            return
        nc.gpsimd.collective_compute(
            "AllGather", mybir.AluOpType.bypass,
            replica_groups=[list(range(NC))], ins=ins, outs=outs,
        )

    xpack_d = nc.dram_tensor("xpack", [BLOCKS, 128, KP], bf16, kind="ExternalInput")
    m1w_d = nc.dram_tensor("m1w", [KP, H], bf16, kind="ExternalInput")
    m2w_d = nc.dram_tensor("m2w", [H + 1, cfg.C], f32, kind="ExternalInput")
    alpha_d = nc.dram_tensor("alpha_s", [128, BLOCKS], f32, kind="ExternalInput")
    iota_d = nc.dram_tensor("iota", [128, 128], f32, kind="ExternalInput")
    ident_d = nc.dram_tensor("ident", [128, 128], f32, kind="ExternalInput")
    nch = plan.nch
    srct_d = nc.dram_tensor("srct", [128, nch], mybir.dt.uint8, kind="ExternalInput")
    we_d = nc.dram_tensor("we", [128, nch], bf16, kind="ExternalInput")
    wo_d = nc.dram_tensor("wo", [128, nch], bf16, kind="ExternalInput")
    idx_d = nc.dram_tensor("idx", [16, plan.ngrp * GCH * 8],
                           mybir.dt.int16, kind="ExternalInput")
    # logits as int8 + the node's bf16 absmax packed in the last two bytes
    out_d = nc.dram_tensor("outp", [NLOC, cfg.C + 2], mybir.dt.int8,
                           kind="ExternalOutput")

    # bf16 replica as [128 partitions, NTOK/128 stripes * 128 values]:
    # token t at (partition t // NSTRIPES, stripe t % NSTRIPES)
    ag_in = nc.dram_tensor("ag_in", [NLOC, H], bf16)
    rep = [
        nc.dram_tensor(f"rep{j}", [128, (NTOK // 128) * 128], bf16,
                       addr_space="Shared")
        for j in range(2)
    ]

    R = [float(v) for v in rcoef]
    off = np.concatenate([[0], np.cumsum(plan.cbs)]).astype(int)

    with tile.TileContext(nc) as tc:
        with (
            tc.tile_pool(name="const", bufs=1) as constp,
            tc.tile_pool(name="xin", bufs=4) as xinp,
            tc.tile_pool(name="msgl", bufs=3) as msglp,
            tc.tile_pool(name="wones", bufs=6) as wp,
            tc.tile_pool(name="upd", bufs=4) as updp,
            tc.tile_pool(name="pub", bufs=2) as pubp,
            tc.tile_pool(name="head", bufs=3) as headp,
            tc.tile_pool(name="psum", bufs=3, space="PSUM") as psump,
            tc.tile_pool(name="psumt", bufs=3, space="PSUM") as psumt,
            tc.tile_pool(name="psumh", bufs=1, space="PSUM") as psumhp,
        ):
            # ---------- resident tiles
            iota_t = constp.tile([128, 128], f32)
            ident_t = constp.tile([128, 128], f32)
            iota16_t = constp.tile([128, 128], bf16)
            ident16_t = constp.tile([128, 128], bf16)
            srct8_t = constp.tile([128, nch], mybir.dt.uint8)
            we16_t = constp.tile([128, nch], bf16)
            wo16_t = constp.tile([128, nch], bf16)
            srct_t = constp.tile([128, nch], f32)
            we_t = constp.tile([128, nch], f32)
            wo_t = constp.tile([128, nch], f32)
            idx_t = constp.tile([128, plan.ngrp * GCH * 8], mybir.dt.int16)
            m2w_t = constp.tile([H + 1, cfg.C], f32)
            alpha_t = constp.tile([128, BLOCKS], f32)
            a05_t = constp.tile([128, BLOCKS], f32)
            x0_t = constp.tile([128, BLOCKS, H], f32)
            y_t = constp.tile([128, BLOCKS, H], f32)
            out_sb = constp.tile([128, BLOCKS, cfg.C + 2], mybir.dt.int8)
            amax_t = constp.tile([128, BLOCKS], f32)
            rcp_t = constp.tile([128, BLOCKS], f32)
            rep_sb = constp.tile([128, (NTOK // 128) * 128], bf16)

            for t, d in [
                (iota_t, iota_d), (ident_t, ident_d), (srct8_t, srct_d),
                (we16_t, we_d), (wo16_t, wo_d), (m2w_t, m2w_d),
                (alpha_t, alpha_d),
            ]:
                nc.sync.dma_start(out=t[:], in_=d[:])
            # gather indices arrive untiled [16, n]; replicate to 128 partitions
            for k in range(8):
                nc.sync.dma_start(out=idx_t[16 * k : 16 * (k + 1), :], in_=idx_d[:])
            # device-side casts: compact uploads -> f32 W-build operands
            nc.vector.tensor_copy(srct_t[:], srct8_t[:])
            nc.vector.tensor_copy(we_t[:], we16_t[:])
            nc.vector.tensor_copy(wo_t[:], wo16_t[:])
            nc.vector.tensor_copy(iota16_t[:], iota_t[:])
            nc.vector.tensor_copy(ident16_t[:], ident_t[:])
            # m1w: KP > 128 partitions -> load as KCH separate [128, H] tiles
            m1w_ts = []
            for kc in range(KCH):
                mt = constp.tile([128, H], bf16, tag=f"m1w{kc}")
                nc.sync.dma_start(out=mt[:], in_=m1w_d[kc * 128 : (kc + 1) * 128, :])
                m1w_ts.append(mt)

            nc.scalar.activation(a05_t[:], alpha_t[:],
                                 mybir.ActivationFunctionType.Sigmoid)
            nc.vector.tensor_scalar_mul(a05_t[:], a05_t[:], 0.5)

            # ---------- encoder: x0 = x @ m1_w + b ; y = r16 * x0
            for b in range(BLOCKS if "e" in phases else 0):
                pe = psump.tile([128, H], f32, tag="acc")
                xt = xinp.tile([128, KP], bf16)
                nc.sync.dma_start(out=xt[:], in_=xpack_d[b])
                for kc in range(KCH):
                    nc.tensor.matmul(pe[:], xt[:, kc * 128 : (kc + 1) * 128],
                                     m1w_ts[kc][:],
                                     start=(kc == 0), stop=(kc == KCH - 1))
                nc.scalar.activation(x0_t[:, b, :], pe[:],
                                     mybir.ActivationFunctionType.Copy)
                nc.vector.tensor_scalar_mul(y_t[:, b, :], pe[:], R[cfg.NITER])

            # publish y (bf16) -> replica 0
            agv = ag_in[:].rearrange("(b p) f -> p b f", p=128)

            def publish(dst_rep):
                yb = pubp.tile([128, BLOCKS, H], bf16, tag="yb")
                nc.vector.tensor_copy(yb[:], y_t[:])
                nc.sync.dma_start(out=agv, in_=yb[:])
                allgather([ag_in[:]], [dst_rep[:]])

            publish(rep[0])

            # ---------- Horner iterations
            nidx_reg = nc.gpsimd.to_reg(GCH * 128)
            for i in range(cfg.NITER if "h" in phases else 0):
                k = cfg.NITER - 1 - i
                # replica HBM -> SBUF: one full-bandwidth contiguous DMA
                nc.sync.dma_start(out=rep_sb[:], in_=rep[i % 2][:])
                # SBUF-source transposed gathers: out [128 feat, edges]
                msg_tiles = []
                for g in range(plan.ngrp):
                    mt = msglp.tile([128, 1, GCH * 128], bf16, tag="msg")
                    for _ in range(RG):
                        nc.gpsimd.dma_gather(
                            mt[:], rep_sb[:],
                            idx_t[:, g * GCH * 8 : (g + 1) * GCH * 8],
                            GCH * 128, nidx_reg, 128, transpose=True,
                            single_packet=False,
                            sbuf_tokens_per_rank=128,
                            sbuf_free_dim_per_rank=256)
                    msg_tiles.append(mt)

                for b in range(BLOCKS):
                    ps = psump.tile([128, H], f32, tag="acc")
                    tot = int(plan.cbs[b])
                    for rm in range(RM):
                        for j in range(tot):
                            col = off[b] + j
                            mt = msg_tiles[col // GCH]
                            cc = col % GCH
                            # chunk back to edge-major via PE transpose
                            pt = psumt.tile([128, 128], bf16, tag="tp")
                            nc.tensor.transpose(
                                pt[:], mt[:, 0, cc * 128 : (cc + 1) * 128],
                                ident16_t[:])
                            me = wp.tile([128, 128], bf16, tag="me")
                            nc.vector.tensor_copy(me[:], pt[:])
                            wte = wp.tile([128, 128], bf16, tag="We")
                            nc.vector.tensor_scalar(
                                wte[:], iota_t[:],
                                srct_t[:, col : col + 1],
                                we_t[:, col : col + 1],
                                mybir.AluOpType.is_equal,
                                mybir.AluOpType.mult)
                            wto = wp.tile([128, 128], bf16, tag="Wo")
                            nc.vector.tensor_scalar(
                                wto[:], iota_t[:],
                                srct_t[:, col : col + 1],
                                wo_t[:, col : col + 1],
                                mybir.AluOpType.is_equal,
                                mybir.AluOpType.mult)
                            nc.tensor.matmul(
                                ps[:], wte[:], me[:, 0:H],
                                start=(j == 0 and rm == 0), stop=False,
                                skip_group_check=True)
                            nc.tensor.matmul(
                                ps[:], wto[:], me[:, H : 2 * H],
                                start=False,
                                stop=(j == tot - 1 and rm == RM - 1),
                                skip_group_check=True)
                    # y' = a05*(az - y) + r_k*x0  == a05*az - (a05*y - r_k*x0)
                    for ru in range(RU):
                        x0s = updp.tile([128, H], f32, tag="x0s")
                        nc.scalar.activation(
                            x0s[:], x0_t[:, b, :],
                            mybir.ActivationFunctionType.Copy, scale=R[k])
                        tt = updp.tile([128, H], f32, tag="tt")
                        nc.vector.scalar_tensor_tensor(
                            tt[:], y_t[:, b, :], a05_t[:, b : b + 1], x0s[:],
                            mybir.AluOpType.mult, mybir.AluOpType.subtract)
                        nc.vector.scalar_tensor_tensor(
                            y_t[:, b, :], ps[:], a05_t[:, b : b + 1], tt[:],
                            mybir.AluOpType.mult, mybir.AluOpType.subtract)

                if i < cfg.NITER - 1:
                    publish(rep[(i + 1) % 2])

            # ---------- head: out = relu(y) @ m2_w + b
            for b in range(BLOCKS if "d" in phases else 0):
                rt = headp.tile([128, H], f32, tag="relu")
                nc.scalar.activation(rt[:], y_t[:, b, :],
                                     mybir.ActivationFunctionType.Relu)
                pt = psumhp.tile([H, 128], f32, tag="tp")
                nc.tensor.transpose(pt[:], rt[:], ident_t[:])
                rta = headp.tile([H + 1, 128], f32, tag="rta")
                nc.vector.memset(rta[H : H + 1, :], 1.0)
                nc.vector.tensor_copy(rta[0:H, :], pt[:])
                po = psumhp.tile([128, cfg.C], f32, tag="po")
                nc.tensor.matmul(po[:], rta[:], m2w_t[:])
                # per-node int8 quantization: i8 = round(po * 127 / absmax)
                nc.vector.tensor_reduce(
                    amax_t[:, b : b + 1], po[:], mybir.AxisListType.X,
                    mybir.AluOpType.max, apply_absolute_value=True)
                nc.vector.tensor_scalar_max(
                    amax_t[:, b : b + 1], amax_t[:, b : b + 1], 1e-30)
                nc.vector.reciprocal(rcp_t[:, b : b + 1], amax_t[:, b : b + 1])
                nc.vector.tensor_scalar(
                    out_sb[:, b, 0 : cfg.C], po[:], rcp_t[:, b : b + 1], 127.0,
                    mybir.AluOpType.mult, mybir.AluOpType.mult)
                nc.vector.tensor_copy(
                    out_sb[:, b, cfg.C : cfg.C + 2].bitcast(bf16),
                    amax_t[:, b : b + 1])

            outv = out_d[:].rearrange("(b p) f -> p b f", p=128)
            nc.sync.dma_start(out=outv, in_=out_sb[:])

    nc.finalize()
    return nc


# ------------------------------------------------------------ cached runner
@dataclass
class RunResults:
    results: list


_RUNNERS: dict = {}       # id(nc) -> (runner tuple, nc)
_DEV_INPUTS: dict = {}    # (id(nc), name) -> (per-core np arrays, device array)
_ZERO_POOL: dict = {}     # id(nc) -> prefetched donated output buffers


def _make_runner(nc, n_cores):
    import jax
    import jax.numpy as jnp
    from jax.sharding import Mesh, PartitionSpec, NamedSharding
    from jax.experimental.shard_map import shard_map
    import concourse.mybir as mybir
    from concourse.bass2jax import (
        _bass_exec_p, fast_dispatch_compile, install_neuronx_cc_hook,
        partition_id_tensor)

    install_neuronx_cc_hook()

    partition_name = (
        nc.partition_id_tensor.name if nc.partition_id_tensor else None)
    in_names, out_names, out_avals, in_avals = [], [], [], []
    for alloc in nc.m.functions[0].allocations:
        if not isinstance(alloc, mybir.MemoryLocationSet):
            continue
        name = alloc.memorylocations[0].name
        if alloc.kind == "ExternalInput":
            if name != partition_name:
                in_names.append(name)
                in_avals.append(jax.core.ShapedArray(
                    tuple(alloc.tensor_shape), mybir.dt.np(alloc.dtype)))
        elif alloc.kind == "ExternalOutput":
            out_names.append(name)
            out_avals.append(jax.core.ShapedArray(
                tuple(alloc.tensor_shape), mybir.dt.np(alloc.dtype)))
    n_params = len(in_names)
    in_names_full = list(in_names) + out_names + (
        [partition_name] if partition_name else [])

    devices = jax.devices()[:n_cores]
    assert len(devices) == n_cores
    mesh = Mesh(np.asarray(devices), ("core",))
    sharding = NamedSharding(mesh, PartitionSpec("core"))

    n_outs = len(out_avals)

    def _body(*args):
        operands = list(args)
        if partition_name is not None:
            operands.append(partition_id_tensor())
        return tuple(_bass_exec_p.bind(
            *operands, out_avals=tuple(out_avals),
            in_names=tuple(in_names_full), out_names=tuple(out_names),
            lowering_input_output_aliases=(), sim_require_finite=True,
            sim_require_nnan=True, nc=nc))

    def _compile_run():
        jitted = jax.jit(shard_map(
            _body, mesh=mesh,
            in_specs=(PartitionSpec("core"),) * (n_params + n_outs),
            out_specs=(PartitionSpec("core"),) * len(out_names),
            check_rep=False),
            donate_argnums=tuple(range(n_params, n_params + n_outs)),
            keep_unused=True)
        arg_structs = [
            jax.ShapeDtypeStruct(
                (n_cores * a.shape[0],) + a.shape[1:], a.dtype,
                sharding=sharding)
            for a in in_avals + out_avals
        ]
        return jitted.lower(*arg_structs).compile()

    # compile with bass_effect suppressed -> C++ fast-path dispatch per call
    run = fast_dispatch_compile(_compile_run)

    # donated output buffers, created on device; prefetched off-critical-path
    zeros_fn = jax.jit(
        lambda: tuple(
            jnp.zeros((n_cores * a.shape[0],) + a.shape[1:], a.dtype)
            for a in out_avals),
        out_shardings=(sharding,) * n_outs)

    def put_sharded(per_core_arrs):
        shards = [np.ascontiguousarray(a) for a in per_core_arrs]
        with ThreadPoolExecutor(n_cores) as ex:
            devs = list(ex.map(
                lambda i: jax.device_put(shards[i], devices[i]),
                range(n_cores)))
        gshape = (sum(s.shape[0] for s in shards),) + shards[0].shape[1:]
        return jax.make_array_from_single_device_arrays(
            gshape, sharding, devs)

    def fetch(out_arrs):
        shard_datas = []
        for arr in out_arrs:
            shard_datas.extend(
                s.data for s in sorted(
                    arr.addressable_shards, key=lambda s: s.index))
        with ThreadPoolExecutor(max(8, len(shard_datas))) as ex:
            fetched = list(ex.map(np.asarray, shard_datas))
        per_out = []
        for j in range(len(out_arrs)):
            per_out.append(fetched[j * n_cores : (j + 1) * n_cores])
        return per_out

    return run, zeros_fn, put_sharded, fetch, in_names, out_names


def run_spmd(nc, in_maps, core_ids) -> RunResults:
    """Drop-in for run_bass_kernel_spmd: executes nc on the first
    len(core_ids) devices, caching the lowered executable and the
    device-resident input buffers across calls."""
    n_cores = len(core_ids)
    key = id(nc)
    if key not in _RUNNERS:
        _RUNNERS[key] = (_make_runner(nc, n_cores), nc)  # keep nc alive
    (run, zeros_fn, put_sharded, fetch, in_names, out_names), _ = _RUNNERS[key]

    dev_in = []
    for name in in_names:
        arrs = [in_maps[c][name] for c in range(n_cores)]
        ck = (key, name)
        hit = _DEV_INPUTS.get(ck)
        if hit is None or any(a is not b for a, b in zip(hit[0], arrs)):
            hit = (arrs, put_sharded(arrs))
            _DEV_INPUTS[ck] = hit
        dev_in.append(hit[1])

    zs = _ZERO_POOL.pop(key, None)
    if zs is None:
        zs = zeros_fn()
    out_arrs = run(*dev_in, *zs)
    _ZERO_POOL[key] = zeros_fn()   # next call's buffers, made while exec runs
    per_out = fetch(out_arrs)
    results = [
        {name: per_out[j][c] for j, name in enumerate(out_names)}
        for c in range(n_cores)
    ]
    return RunResults(results=results)


# ------------------------------------------------------------ entry point
def assemble_output(res: RunResults, cfg: Cfg) -> np.ndarray:
    """Dequantize per-core int8 logits; the node's bf16 absmax rides in the
    last two bytes of each row."""
    outs = []
    for c in range(cfg.NCORES):
        raw = res.results[c]["outp"][: cfg.NSH]
        i8 = raw[:, : cfg.C].astype(np.float32)
        amax = np.ascontiguousarray(raw[:, cfg.C :]).view(BF16).astype(np.float32)
        outs.append(i8 * (amax * (1.0 / 127.0)))
    return np.concatenate(outs, axis=0).astype(np.float32)


def kernel(**inputs) -> np.ndarray:
    cfg = Cfg()
    rcoef = horner_coeffs(cfg)
    plan = build_plan(cfg, inputs)
    nc = build_program(cfg, plan, rcoef)

    res = run_spmd(nc, plan.in_maps, list(range(cfg.NCORES)))
    return assemble_output(res, cfg)
